# revision 1
# baseline (speedup 1.0000x reference)
"""2-layer GAT on 8 Trainium2 NeuronCores (Bass/Tile).

Strategy (dst-partitioned graph parallelism):
  Host: add self-loops, sort edges by dst, pack dsts into 392 degree-balanced
  bins of 128 (256 bins hold nodes < 32768, 136 bins hold the rest, so every
  bin is homogeneous w.r.t. the int16-index table half it needs). 8 cores x
  49 bins each (32 lo + 17 hi), every bin padded to a uniform edge capacity so
  all cores run the same program (SPMD: one NEFF, per-core data).

  NEFF-A: per core, g[n] = x[n] @ [W1 | W1@As | W1@Ad] for 1/8 of nodes.
  NEFF-B: per core, layer-1 edge phase over its bins: dma_gather of packed
    rows g[src] (h|a_src) and g[dst] (a_dst half), per-edge attention
    w = exp(leaky(t)) = max(exp(t-M), exp(0.2t-M)), segment softmax +
    aggregation as one fp16 matmul per 128-edge chunk against a selection
    matrix S[e,j] = (dstloc[e]==j), accumulating [out1 | denom] in PSUM per
    bin; then ELU + @[W2 | W2@as2 | W2@ad2] -> g2 rows.
  NEFF-C: per core, layer-2 edge phase, same machinery with 18-wide rows.

  Host relays the small g / g2 tables between the three launches.
"""

import sys
import time

sys.path.insert(0, "/opt/trn_rl_repo")

import numpy as np

import concourse.bacc as bacc
import concourse.bass as bass
import concourse.mybir as mybir
import concourse.tile as tile
from concourse import bass2jax

# ---------------- problem constants (hardcoded per task contract) -------------
N = 50000
F_IN = 128
HID = 16
HEADS = 8
CLASSES = 16
NEG = 0.2

N_CORES = 8
P = 128
BLOCKS_PER_CORE = 49
NBINS = N_CORES * BLOCKS_PER_CORE          # 392
N_PAD = NBINS * P                          # 50176
LO_N = 32768                               # table half split (int16 indices)
LO_BINS = LO_N // P                        # 256  -> 32 per core
HI_BINS = NBINS - LO_BINS                  # 136  -> 17 per core
LO_BLOCKS_PER_CORE = LO_BINS // N_CORES    # 32
NODES_PER_CORE = N_PAD // N_CORES          # 6272

GROW = 256                                 # g row elems (fp16) = 512B
G2ROW = 128                                # g2 row elems (fp16) = 256B
M_SHIFT = 4.0                              # softmax-invariant exp shift
GMAX = 1024                                # max indices per dma_gather (ring cap)
NQ = 4                                     # SWDGE queues

F16 = mybir.dt.float16
F32 = mybir.dt.float32
I16 = mybir.dt.int16

_cache = {}
_last_cfg = None
_last_inputs = None


# ---------------------------- host preprocessing -----------------------------

def _binpack(nodes, deg, nbins):
    """Greedy balanced packing of `nodes` into nbins bins of exactly P each."""
    import heapq
    order = np.argsort(-deg[nodes], kind="stable")
    nodes = nodes[order]
    heap = [(0, 0, b) for b in range(nbins)]
    heapq.heapify(heap)
    members = np.empty((nbins, P), np.int64)
    counts = np.zeros(nbins, np.int32)
    sums = np.zeros(nbins, np.int64)
    for nd in nodes:
        d = int(deg[nd])
        stash = []
        while True:
            s, c, b = heapq.heappop(heap)
            if c < P:
                members[b, c] = nd
                counts[b] += 1
                sums[b] += d
                heapq.heappush(heap, (s + d, c + 1, b))
                break
            stash.append((s, c, b))
        for t in stash:
            heapq.heappush(heap, t)
    assert (counts == P).all()
    return members, sums


def _wrap16(a):
    """Index array [n] -> dma_gather SBUF layout [128, n/16] (i at [i%16,i//16],
    replicated through all partitions so any SWDGE queue can read it)."""
    n = a.shape[0]
    assert n % 16 == 0
    w = a.reshape(n // 16, 16).T.astype(np.int16)
    return np.tile(w, (8, 1))


def host_prep(edge_index):
    src = np.concatenate([edge_index[0].astype(np.int64),
                          np.arange(N, dtype=np.int64)])
    dst = np.concatenate([edge_index[1].astype(np.int64),
                          np.arange(N, dtype=np.int64)])
    deg = np.bincount(dst, minlength=N_PAD)

    order = np.argsort(dst, kind="stable")
    src_s = src[order]
    estart = np.zeros(N_PAD + 1, np.int64)
    np.cumsum(deg, out=estart[1:])

    lo_nodes = np.arange(LO_N)
    hi_nodes = np.arange(LO_N, N_PAD)
    mem_lo, sum_lo = _binpack(lo_nodes, deg, LO_BINS)
    mem_hi, sum_hi = _binpack(hi_nodes, deg, HI_BINS)

    # core k owns lo bins [k*32,(k+1)*32) and hi bins [k*17,(k+1)*17)
    # program order per core: 32 lo blocks then 17 hi blocks
    bin_of = np.empty((N_CORES, BLOCKS_PER_CORE, P), np.int64)
    for k in range(N_CORES):
        bin_of[k, :LO_BLOCKS_PER_CORE] = mem_lo[k * LO_BLOCKS_PER_CORE:(k + 1) * LO_BLOCKS_PER_CORE]
        bin_of[k, LO_BLOCKS_PER_CORE:] = mem_hi[k * (BLOCKS_PER_CORE - LO_BLOCKS_PER_CORE):(k + 1) * (BLOCKS_PER_CORE - LO_BLOCKS_PER_CORE)]

    # per-(core,block) lo/hi-src slot counts -> uniform section caps
    max_lo = 0
    max_hi = 0
    block_edges = []
    for k in range(N_CORES):
        per_core = []
        for b in range(BLOCKS_PER_CORE):
            es = []
            for j in range(P):
                nd = bin_of[k, b, j]
                s, e = estart[nd], estart[nd + 1]
                es.append((src_s[s:e], j))
            lo_cnt = sum((a < LO_N).sum() for a, _ in es)
            hi_cnt = sum((a >= LO_N).sum() for a, _ in es)
            max_lo = max(max_lo, int(lo_cnt))
            max_hi = max(max_hi, int(hi_cnt))
            per_core.append(es)
        block_edges.append(per_core)

    L_cap = -(-max_lo // P) * P
    H_cap = -(-max_hi // P) * P
    cap = L_cap + H_cap
    chunks = cap // P

    si16 = np.zeros((N_CORES, BLOCKS_PER_CORE, 128, cap // 16), np.int16)
    di16 = np.zeros((N_CORES, BLOCKS_PER_CORE, 128, cap // 16), np.int16)
    dl = np.full((N_CORES, BLOCKS_PER_CORE, P, chunks), -1.0, np.float32)

    for k in range(N_CORES):
        for b in range(BLOCKS_PER_CORE):
            slo, shi, jlo, jhi = [], [], [], []
            for arr, j in block_edges[k][b]:
                lo_m = arr < LO_N
                slo.append(arr[lo_m]); shi.append(arr[~lo_m])
                jlo.append(np.full(lo_m.sum(), j)); jhi.append(np.full((~lo_m).sum(), j))
            slo = np.concatenate(slo); shi = np.concatenate(shi)
            jlo = np.concatenate(jlo); jhi = np.concatenate(jhi)
            # pad sections (idx 0 gathers a real row; dstloc -1 masks it out)
            s_lo = np.zeros(L_cap, np.int64); s_lo[:len(slo)] = slo
            s_hi = np.zeros(H_cap, np.int64); s_hi[:len(shi)] = shi - LO_N
            j_all = np.full(cap, -1.0, np.float32)
            j_all[:len(jlo)] = jlo
            j_all[L_cap:L_cap + len(jhi)] = jhi
            d_all = np.zeros(cap, np.int64)
            d_all[:len(jlo)] = bin_of[k, b][jlo]
            d_all[L_cap:L_cap + len(jhi)] = bin_of[k, b][jhi]
            if b >= LO_BLOCKS_PER_CORE:
                d_all = d_all - LO_N          # hi block: dst table is g_hi
                d_all[j_all < 0] = 0
            si16[k, b] = np.concatenate([_wrap16(s_lo), _wrap16(s_hi)], axis=1)
            di16[k, b] = _wrap16(d_all)
            dl[k, b] = j_all.reshape(chunks, P).T

    return dict(bin_of=bin_of, si16=si16, di16=di16, dl=dl,
                L_cap=L_cap, H_cap=H_cap, cap=cap, chunks=chunks)


# ------------------------------- NEFF builders -------------------------------

def build_neff_a(reps=1):
    nc = bacc.Bacc()
    xT = nc.dram_tensor("xT", [P, NODES_PER_CORE], F32, kind="ExternalInput")
    w1e = nc.dram_tensor("w1e", [P, 144], F32, kind="ExternalInput")
    g_out = nc.dram_tensor("g_out", [NODES_PER_CORE, 144], F16, kind="ExternalOutput")
    ntiles = NODES_PER_CORE // P
    with tile.TileContext(nc) as tc:
        with tc.tile_pool(name="sbuf", bufs=4) as pool, \
             tc.tile_pool(name="psum", bufs=4, space="PSUM") as pp:
            w1t = pool.tile([P, 144], F32)
            nc.sync.dma_start(w1t[:], w1e[:])

            def body():
                for t in range(ntiles):
                    xt = pool.tile([P, P], F32, tag="xt", name="xt")
                    nc.sync.dma_start(xt[:], xT[:, t * P:(t + 1) * P])
                    ps = pp.tile([P, 144], F32, tag="ps", space="PSUM", name="ps")
                    nc.tensor.matmul(out=ps[:], lhsT=xt[:], rhs=w1t[:], start=True, stop=True)
                    gt = pool.tile([P, 144], F16, tag="gt", name="gt")
                    nc.vector.tensor_copy(out=gt[:], in_=ps[:])
                    nc.sync.dma_start(g_out[t * P:(t + 1) * P, :], gt[:])

            if reps == 1:
                body()
            else:
                with tc.For_i(0, reps, 1):
                    body()
    nc.finalize()
    return nc


def _edge_layer(nc, tc, *, cfg, g_d, si_d, di_d, dl_d, iota_d, consts, row,
                nsec, fdim, alpha_cols, out_cb, reps=1):
    """Shared edge-phase skeleton for NEFF-B / NEFF-C.

    row: table row elems (fp16); fdim: feature cols used (128 or 16);
    alpha_cols: (src_col, dst_col) within the gathered rows;
    out_cb(tc, pool, pp, blk, acc_psum, consts): consumes [P, fdim+nh] PSUM.
    """
    cap, chunks, L_cap = cfg["cap"], cfg["chunks"], cfg["L_cap"]
    Lch = L_cap // P
    qctr = [0]

    def qrr():
        qctr[0] = (qctr[0] + 1) % NQ
        return qctr[0]
    nh = HEADS if fdim == P else 1
    rhs_w = fdim + nh
    with tc.tile_pool(name="sbuf", bufs=3) as pool, \
         tc.tile_pool(name="psum", bufs=2, space="PSUM") as pp:
        iota_t = pool.tile([P, P], F32)
        nc.sync.dma_start(iota_t[:], iota_d[:])
        mshift = pool.tile([P, 1], F32)
        nc.gpsimd.memset(mshift[:], -M_SHIFT)
        cl = {}
        for nm, d in consts.items():
            cl[nm] = pool.tile([P, d.shape[1]], F32, tag=f"c_{nm}", name=f"c_{nm}")
            nc.sync.dma_start(cl[nm][:], d[:, :])
        from contextlib import nullcontext

        def body():
            for blk in range(BLOCKS_PER_CORE):
                hi = blk >= LO_BLOCKS_PER_CORE
                g_lo = g_d[0:LO_N, :]
                g_hi = g_d[LO_N:N_PAD, :]
                si = pool.tile([P, cap // 16], I16, tag="si")
                nc.sync.dma_start(si[:, :], si_d[blk])
                di = pool.tile([P, cap // 16], I16, tag="di")
                nc.sync.dma_start(di[:, :], di_d[blk])
                dlt = pool.tile([P, chunks], F32, tag="dl")
                nc.sync.dma_start(dlt[:], dl_d[blk])

                X = pool.tile([P, chunks, row], F16, tag="X")
                for lo0 in range(0, L_cap, GMAX):
                    n = min(GMAX, L_cap - lo0)
                    nc.gpsimd.dma_gather(
                        out_ap=X[:, lo0 // P:(lo0 + n) // P, :], in_ap=g_lo,
                        idxs_ap=si[:, lo0 // 16:(lo0 + n) // 16],
                        num_idxs=n, num_idxs_reg=n, elem_size=row,
                        queue_num=qrr())
                for hi0 in range(0, cap - L_cap, GMAX):
                    n = min(GMAX, cap - L_cap - hi0)
                    nc.gpsimd.dma_gather(
                        out_ap=X[:, Lch + hi0 // P:Lch + (hi0 + n) // P, :], in_ap=g_hi,
                        idxs_ap=si[:, (L_cap + hi0) // 16:(L_cap + hi0 + n) // 16],
                        num_idxs=n, num_idxs_reg=n, elem_size=row,
                        queue_num=qrr())
                # dst-side rows: only the alpha half (256B) is needed
                adw = row if row == G2ROW else row // 2
                ad_base = 0 if row == G2ROW else row // 2
                g_ad = (g_d[LO_N:N_PAD, ad_base:ad_base + adw] if hi
                        else g_d[0:LO_N, ad_base:ad_base + adw])
                AD = pool.tile([P, chunks, adw], F16, tag="AD")
                for d0 in range(0, cap, GMAX):
                    n = min(GMAX, cap - d0)
                    nc.gpsimd.dma_gather(
                        out_ap=AD[:, d0 // P:(d0 + n) // P, :], in_ap=g_ad,
                        idxs_ap=di[:, d0 // 16:(d0 + n) // 16],
                        num_idxs=n, num_idxs_reg=n, elem_size=adw, elem_step=row,
                        queue_num=qrr())

                S = pool.tile([P, chunks, P], F16, tag="S")
                nc.vector.tensor_tensor(
                    out=S[:], in0=iota_t[:, None, :].to_broadcast([P, chunks, P]),
                    in1=dlt[:, :, None].to_broadcast([P, chunks, P]),
                    op=mybir.AluOpType.is_equal)

                sc, dc = alpha_cols
                dca = dc - ad_base
                t_t = pool.tile([P, chunks, nh], F32, tag="t")
                nc.vector.tensor_tensor(out=t_t[:], in0=X[:, :, sc:sc + nh],
                                        in1=AD[:, :, dca:dca + nh], op=mybir.AluOpType.add)
                e1 = pool.tile([P, chunks, nh], F16, tag="e1")
                nc.scalar.activation(e1[:], t_t[:], mybir.ActivationFunctionType.Exp,
                                     bias=mshift[:])
                e2 = pool.tile([P, chunks, nh], F16, tag="e2")
                nc.scalar.activation(e2[:], t_t[:], mybir.ActivationFunctionType.Exp,
                                     bias=mshift[:], scale=NEG)
                we = pool.tile([P, chunks, nh], F16, tag="we")
                nc.vector.tensor_tensor(out=we[:], in0=e1[:], in1=e2[:],
                                        op=mybir.AluOpType.max)

                rhs = pool.tile([P, chunks, rhs_w], F16, tag="rhs")
                kk = fdim // nh
                nc.vector.tensor_tensor(
                    out=rhs[:, :, 0:fdim].rearrange("p c (h k) -> p c h k", k=kk),
                    in0=X[:, :, 0:fdim].rearrange("p c (h k) -> p c h k", k=kk),
                    in1=we[:, :, :, None].to_broadcast([P, chunks, nh, kk]),
                    op=mybir.AluOpType.mult)
                nc.vector.tensor_copy(out=rhs[:, :, fdim:rhs_w], in_=we[:])

                acc = pp.tile([P, rhs_w], F32, tag="acc", space="PSUM")
                for c in range(chunks):
                    nc.tensor.matmul(out=acc[:], lhsT=S[:, c, :], rhs=rhs[:, c, :],
                                     start=(c == 0), stop=(c == chunks - 1))
                out_cb(pool, pp, blk, acc, cl, 0)

        if reps == 1:
            body()
        else:
            with tc.For_i(0, reps, 1):
                body()


def build_neff_b(cfg, reps=1):
    nc = bacc.Bacc(num_swdge_queues=NQ)
    cap, chunks = cfg["cap"], cfg["chunks"]
    g_d = nc.dram_tensor("g", [N_PAD, GROW], F16, kind="ExternalInput")
    si_d = nc.dram_tensor("si", [BLOCKS_PER_CORE, 128, cap // 16], I16, kind="ExternalInput")
    di_d = nc.dram_tensor("di", [BLOCKS_PER_CORE, 128, cap // 16], I16, kind="ExternalInput")
    dl_d = nc.dram_tensor("dl", [BLOCKS_PER_CORE, P, chunks], F32, kind="ExternalInput")
    iota_d = nc.dram_tensor("iota", [P, P], F32, kind="ExternalInput")
    ident_d = nc.dram_tensor("ident", [P, P], F32, kind="ExternalInput")
    w2e_d = nc.dram_tensor("w2e", [P, 18], F32, kind="ExternalInput")
    b1_d = nc.dram_tensor("b1r", [P, P], F32, kind="ExternalInput")
    g2_out = nc.dram_tensor("g2_out", [BLOCKS_PER_CORE, P, 18], F16, kind="ExternalOutput")

    def out_cb(pool, pp, blk, acc, cl, rep):
        recip = pool.tile([P, HEADS], F32, tag="recip")
        nc.vector.reciprocal(recip[:], acc[:, P:P + HEADS])
        o1 = pool.tile([P, P], F32, tag="o1")
        nc.vector.tensor_tensor(
            out=o1[:].rearrange("p (h k) -> p h k", k=HID),
            in0=acc[:, 0:P].rearrange("p (h k) -> p h k", k=HID),
            in1=recip[:, :, None].to_broadcast([P, HEADS, HID]),
            op=mybir.AluOpType.mult)
        nc.vector.tensor_tensor(out=o1[:], in0=o1[:], in1=cl["b1r"][:],
                                op=mybir.AluOpType.add)
        vmin = pool.tile([P, P], F32, tag="vmin")
        nc.vector.tensor_scalar(out=vmin[:], in0=o1[:], scalar1=0.0, scalar2=None,
                                op0=mybir.AluOpType.min)
        ev = pool.tile([P, P], F32, tag="ev")
        nc.scalar.activation(ev[:], vmin[:], mybir.ActivationFunctionType.Exp)
        elu = pool.tile([P, P], F32, tag="elu")
        nc.vector.tensor_scalar(out=elu[:], in0=o1[:], scalar1=0.0, scalar2=None,
                                op0=mybir.AluOpType.max)
        nc.vector.tensor_tensor(out=elu[:], in0=elu[:], in1=ev[:], op=mybir.AluOpType.add)
        nc.vector.tensor_scalar(out=elu[:], in0=elu[:], scalar1=-1.0, scalar2=None,
                                op0=mybir.AluOpType.add)
        eTp = pp.tile([P, P], F32, tag="eTp", space="PSUM")
        nc.tensor.transpose(out=eTp[:], in_=elu[:], identity=cl["ident"][:])
        eT = pool.tile([P, P], F32, tag="eT")
        nc.vector.tensor_copy(out=eT[:], in_=eTp[:])
        g2p = pp.tile([P, 18], F32, tag="g2p", space="PSUM")
        nc.tensor.matmul(out=g2p[:], lhsT=eT[:], rhs=cl["w2e"][:], start=True, stop=True)
        g2t = pool.tile([P, 18], F16, tag="g2t")
        nc.vector.tensor_copy(out=g2t[:], in_=g2p[:])
        nc.sync.dma_start(g2_out[blk], g2t[:])

    with tile.TileContext(nc) as tc:
        _edge_layer(nc, tc, cfg=cfg, g_d=g_d, si_d=si_d, di_d=di_d, dl_d=dl_d,
                    iota_d=iota_d,
                    consts={"ident": ident_d, "w2e": w2e_d, "b1r": b1_d},
                    row=GROW, nsec=2, fdim=P, alpha_cols=(128, 136),
                    out_cb=out_cb, reps=reps)
    nc.finalize()
    return nc


def build_neff_c(cfg, reps=1):
    nc = bacc.Bacc(num_swdge_queues=NQ)
    cap, chunks = cfg["cap"], cfg["chunks"]
    g_d = nc.dram_tensor("g2", [N_PAD, G2ROW], F16, kind="ExternalInput")
    si_d = nc.dram_tensor("si", [BLOCKS_PER_CORE, 128, cap // 16], I16, kind="ExternalInput")
    di_d = nc.dram_tensor("di", [BLOCKS_PER_CORE, 128, cap // 16], I16, kind="ExternalInput")
    dl_d = nc.dram_tensor("dl", [BLOCKS_PER_CORE, P, chunks], F32, kind="ExternalInput")
    iota_d = nc.dram_tensor("iota", [P, P], F32, kind="ExternalInput")
    b2_d = nc.dram_tensor("b2r", [P, CLASSES], F32, kind="ExternalInput")
    out_d = nc.dram_tensor("out2", [BLOCKS_PER_CORE, P, CLASSES], F32, kind="ExternalOutput")

    def out_cb(pool, pp, blk, acc, cl, rep):
        recip = pool.tile([P, 1], F32, tag="recip")
        nc.vector.reciprocal(recip[:], acc[:, CLASSES:CLASSES + 1])
        o2 = pool.tile([P, CLASSES], F32, tag="o2")
        nc.vector.tensor_tensor(out=o2[:], in0=acc[:, 0:CLASSES],
                                in1=recip[:].to_broadcast([P, CLASSES]),
                                op=mybir.AluOpType.mult)
        nc.vector.tensor_tensor(out=o2[:], in0=o2[:], in1=cl["b2r"][:],
                                op=mybir.AluOpType.add)
        nc.sync.dma_start(out_d[blk], o2[:])

    with tile.TileContext(nc) as tc:
        _edge_layer(nc, tc, cfg=cfg, g_d=g_d, si_d=si_d, di_d=di_d, dl_d=dl_d,
                    iota_d=iota_d, consts={"b2r": b2_d},
                    row=G2ROW, nsec=2, fdim=CLASSES, alpha_cols=(16, 17),
                    out_cb=out_cb, reps=reps)
    nc.finalize()
    return nc


# ------------------------------ runner plumbing ------------------------------

def make_runner(nc, n_cores=N_CORES):
    """Cached shard_map runner for a finalized Bass module. Returns
    run(in_maps) -> list of per-core output dicts."""
    import jax
    from jax.sharding import Mesh, PartitionSpec
    from jax.experimental.shard_map import shard_map
    from concourse.bass2jax import _bass_exec_p, install_neuronx_cc_hook, partition_id_tensor

    install_neuronx_cc_hook()
    partition_name = nc.partition_id_tensor.name if nc.partition_id_tensor else None
    in_names, out_names, out_avals = [], [], []
    for alloc in nc.m.functions[0].allocations:
        if not isinstance(alloc, mybir.MemoryLocationSet):
            continue
        name = alloc.memorylocations[0].name
        if alloc.kind == "ExternalInput":
            if name != partition_name:
                in_names.append(name)
        elif alloc.kind == "ExternalOutput":
            out_names.append(name)
            out_avals.append(jax.core.ShapedArray(tuple(alloc.tensor_shape),
                                                  mybir.dt.np(alloc.dtype)))
    n_params = len(in_names)
    all_names = in_names + out_names + ([partition_name] if partition_name else [])

    def _body(*args):
        operands = list(args)
        if partition_name is not None:
            operands.append(partition_id_tensor())
        return tuple(_bass_exec_p.bind(
            *operands, out_avals=tuple(out_avals), in_names=tuple(all_names),
            out_names=tuple(out_names), lowering_input_output_aliases=(),
            sim_require_finite=False, sim_require_nnan=False, nc=nc))

    devices = jax.devices()[:n_cores]
    mesh = Mesh(np.asarray(devices), ("core",))
    sharded = jax.jit(
        shard_map(_body, mesh=mesh,
                  in_specs=(PartitionSpec("core"),) * (n_params + len(out_names)),
                  out_specs=(PartitionSpec("core"),) * len(out_names),
                  check_rep=False),
        keep_unused=True)

    import jax as _jax
    from jax.sharding import NamedSharding

    _dev_args = {}

    def run(in_maps, key=None, raw=False):
        if key is not None and key in _dev_args:
            args = _dev_args[key]
        else:
            concat_in = [np.concatenate([np.asarray(m[nm]) for m in in_maps], axis=0)
                         for nm in in_names]
            concat_zero = [np.zeros((n_cores * a.shape[0], *a.shape[1:]), a.dtype)
                           for a in out_avals]
            sh = NamedSharding(mesh, PartitionSpec("core"))
            args = [_jax.device_put(a, sh) for a in concat_in + concat_zero]
            _jax.block_until_ready(args)
            if key is not None:
                _dev_args[key] = args
        outs = sharded(*args)
        _jax.block_until_ready(outs)
        if raw:
            return outs
        return [
            {nm: np.asarray(outs[i]).reshape(n_cores, *out_avals[i].shape)[c]
             for i, nm in enumerate(out_names)}
            for c in range(n_cores)
        ]

    return run


def _get_compiled(key, builder):
    if key not in _cache:
        nc = builder()
        _cache[key] = make_runner(nc)
    return _cache[key]


# --------------------------------- kernel ------------------------------------

def kernel(x, edge_index, W1, a_src1, a_dst1, b1, W2, a_src2, a_dst2, b2):
    x = np.asarray(x, np.float32)
    edge_index = np.asarray(edge_index)
    W1 = np.asarray(W1, np.float32)
    W2 = np.asarray(W2, np.float32)
    a_src1 = np.asarray(a_src1, np.float32)
    a_dst1 = np.asarray(a_dst1, np.float32)
    a_src2 = np.asarray(a_src2, np.float32)
    a_dst2 = np.asarray(a_dst2, np.float32)
    b1 = np.asarray(b1, np.float32)
    b2 = np.asarray(b2, np.float32)

    cfg = host_prep(edge_index)

    As = np.zeros((P, HEADS), np.float32)
    Ad = np.zeros((P, HEADS), np.float32)
    for h in range(HEADS):
        As[h * HID:(h + 1) * HID, h] = a_src1[h]
        Ad[h * HID:(h + 1) * HID, h] = a_dst1[h]
    W1ext = np.concatenate([W1, W1 @ As, W1 @ Ad], 1).astype(np.float32)
    W2ext = np.concatenate([W2, W2 @ a_src2.T, W2 @ a_dst2.T], 1).astype(np.float32)
    iota = np.ascontiguousarray(np.broadcast_to(np.arange(P, dtype=np.float32), (P, P)))
    ident = np.eye(P, dtype=np.float32)
    b1r = np.ascontiguousarray(np.broadcast_to(b1, (P, P))).astype(np.float32)
    b2r = np.ascontiguousarray(np.broadcast_to(b2, (P, CLASSES))).astype(np.float32)

    xT = np.zeros((P, N_PAD), np.float32)
    xT[:, :N] = x.T

    # ---- NEFF-A ----
    run_a = _get_compiled("A", build_neff_a)
    in_a = [{"xT": np.ascontiguousarray(xT[:, k * NODES_PER_CORE:(k + 1) * NODES_PER_CORE]),
             "w1e": W1ext} for k in range(N_CORES)]
    res_a = run_a(in_a)
    g_full = np.zeros((N_PAD, GROW), np.float16)
    for k in range(N_CORES):
        g_full[k * NODES_PER_CORE:(k + 1) * NODES_PER_CORE, 0:144] = res_a[k]["g_out"]

    # ---- NEFF-B ----
    run_b = _get_compiled(("B", cfg["cap"]), lambda: build_neff_b(cfg))
    in_b = [{"g": g_full, "si": cfg["si16"][k], "di": cfg["di16"][k],
             "dl": cfg["dl"][k], "iota": iota, "ident": ident, "w2e": W2ext,
             "b1r": b1r} for k in range(N_CORES)]
    res_b = run_b(in_b)
    g2_full = np.zeros((N_PAD, G2ROW), np.float16)
    for k in range(N_CORES):
        rows = cfg["bin_of"][k].reshape(-1)            # [49*128] node ids
        g2_full[rows, 0:18] = res_b[k]["g2_out"].reshape(-1, 18)
    g2_full[np.isnan(g2_full.astype(np.float32)).any(1)] = 0   # pad-node rows

    # ---- NEFF-C ----
    run_c = _get_compiled(("C", cfg["cap"]), lambda: build_neff_c(cfg))
    in_c = [{"g2": g2_full, "si": cfg["si16"][k], "di": cfg["di16"][k],
             "dl": cfg["dl"][k], "iota": iota, "b2r": b2r} for k in range(N_CORES)]
    res_c = run_c(in_c)

    out = np.zeros((N_PAD, CLASSES), np.float32)
    for k in range(N_CORES):
        rows = cfg["bin_of"][k].reshape(-1)
        out[rows] = res_c[k]["out2"].reshape(-1, CLASSES)

    global _last_cfg, _last_inputs
    _last_cfg = cfg
    _last_inputs = {"A": in_a, "B": in_b, "C": in_c}
    return out[:N].astype(np.float32)



# revision 10
# speedup vs baseline: 2.5170x; 2.5170x over previous
"""2-layer GAT on 8 Trainium2 NeuronCores (Bass/Tile) — V2.

Strategy (dst-per-partition graph parallelism):
  Host: sort nodes by degree; 392 bins of 128 nodes; partition p of a bin owns
  dst node bin[p] and ALL its incoming edges live in partition p's chunk row.
  Per-dst alpha_dst then broadcasts along the free dim (no per-edge dst
  gather), the segment softmax denominator and the aggregation are free-dim
  tree reductions on DVE (no selection-matrix matmuls).

  NEFF-A: g[n] = x[n] @ [W1 | W1@As | W1@Ad] for 1/8 of nodes (one big load,
    one big store).
  NEFF-B: layer-1 edge phase. g table rows 512B fp16 ([h 128 | as 8 | ad 8 |
    pad]). int16 gather indices cover 32768 rows, so two overlapping views
    (rows 0..32767 and rows 17408..50175) are used; per-node mid-range srcs
    (17408..32767) are assigned to either call to balance the per-bin lo/hi
    section sizes. 2 dma_gather calls per bin; dst rows ride along as 2 extra
    chunks. Per-edge w = exp(leaky(as_src+ad_dst))*e^-4 via two exps + max;
    out rows ELU'd and pushed through [W2 | W2@as2 | W2@ad2].
  NEFF-C: layer-2 edge phase over a pair-packed table (2 nodes per 256B unit,
    64-elem sub-rows) -> single table, 1 gather call per bin; per-slot parity
    masks select the sub-row.

  8 cores x 49 bins each; bins are rank-grouped in 8s (one per core) with
  shared shapes so all cores run one NEFF (SPMD).
"""

import sys

sys.path.insert(0, "/opt/trn_rl_repo")

import numpy as np

import concourse.bacc as bacc
import concourse.bass as bass
import concourse.mybir as mybir
import concourse.tile as tile

# ---------------- problem constants (hardcoded per task contract) -------------
N = 50000
F_IN = 128
HID = 16
HEADS = 8
CLASSES = 16
NEG = 0.2

N_CORES = 8
P = 128
BLOCKS_PER_CORE = 49
NBINS = N_CORES * BLOCKS_PER_CORE          # 392
N_PAD = NBINS * P                          # 50176
NODES_PER_CORE = N_PAD // N_CORES          # 6272

LOA = 32768                                # table view A = rows [0, 32768)
MIDBASE = 17408                            # table view B = rows [17408, 50176)

GROW = 256                                 # g row elems (fp16) = 512B
G2SUB = 64                                 # g2 sub-row elems (fp16) = 128B
G2UNIT = 128                               # g2 unit elems (2 nodes) = 256B
NUNIT = N_PAD // 2                         # 25088 pair units
M_SHIFT = 4.0                              # softmax-invariant exp shift
GMAX = 1024                                # max indices per dma_gather call
NQ = 4                                     # SWDGE queues

F16 = mybir.dt.float16
F32 = mybir.dt.float32
I16 = mybir.dt.int16

_cache = {}
_last_cfg = None
_last_inputs = None


# ---------------------------- host preprocessing -----------------------------

def _wrap16(a):
    """Index array [n] -> dma_gather SBUF layout [128, n/16]."""
    n = a.shape[0]
    assert n % 16 == 0
    w = a.reshape(n // 16, 16).T.astype(np.int16)
    return np.tile(w, (8, 1))


def _group_shapes_b(lmin, m, hmin, bins):
    """Per 8-bin rank group: single (L, H) so every core runs one program.
    L/H = lo/hi edge-chunk counts; mid edges flex between the two calls."""
    Ls, Hs = [], []
    for g in range(BLOCKS_PER_CORE):
        nodes = bins[g * 8:(g + 1) * 8].reshape(-1)
        lo, mi, hi = lmin[nodes], m[nodes], hmin[nodes]
        best = None
        for L in range(int(lo.max()), int((lo + mi).max()) + 1):
            H = int((hi + np.maximum(0, mi - (L - lo))).max())
            if best is None or L + H < best[0]:
                best = (L + H, L, H)
        Ls.append(best[1])
        Hs.append(best[2])
    return Ls, Hs


def host_prep(edge_index):
    src = np.concatenate([edge_index[0].astype(np.int64),
                          np.arange(N, dtype=np.int64)])
    dst = np.concatenate([edge_index[1].astype(np.int64),
                          np.arange(N, dtype=np.int64)])
    order = np.argsort(dst, kind="stable")
    src_s = src[order]
    deg = np.bincount(dst, minlength=N_PAD)
    estart = np.zeros(N_PAD + 1, np.int64)
    np.cumsum(deg, out=estart[1:])

    lmin = np.bincount(dst[src < MIDBASE], minlength=N_PAD)
    hmin = np.bincount(dst[src >= LOA], minlength=N_PAD)
    m = deg - lmin - hmin

    # ---- B binning: sort by (deg, lmin); bins of 128; groups of 8 bins ----
    nodesB = np.lexsort((lmin, deg))
    binsB = nodesB.reshape(NBINS, P)
    LsB, HsB = _group_shapes_b(lmin, m, hmin, binsB)

    # per-bin slot tables. X chunk layout: [0,L) lo edges | L = dstA | L+1 =
    # dstB | [L+2, L+2+H) hi edges.  D = L+H+2.
    siB_parts, mB_parts = [], []
    for k in range(N_CORES):
        si_list, m_list = [], []
        for j in range(BLOCKS_PER_CORE):
            L, H = LsB[j], HsB[j]
            D = L + H + 2
            nodes = binsB[j * 8 + k]
            Alo = np.zeros((L + 1, P), np.int64)   # chunks 0..L (incl dstA)
            Ahi = np.zeros((H + 1, P), np.int64)   # chunks L+1..L+1+H (dstB first)
            msk = np.zeros((P, D + 2), np.float32)  # D edge/dst cols + adm + pad
            for p in range(P):
                nd = nodes[p]
                es = src_s[estart[nd]:estart[nd + 1]]
                elo = es[es < MIDBASE]
                emid = es[(es >= MIDBASE) & (es < LOA)]
                ehi = es[es >= LOA]
                x = min(len(emid), L - len(elo))
                lo_list = np.concatenate([elo, emid[:x]])
                hi_list = np.concatenate([emid[x:], ehi])
                assert len(lo_list) <= L and len(hi_list) <= H
                Alo[:len(lo_list), p] = lo_list
                Ahi[1:1 + len(hi_list), p] = hi_list - MIDBASE
                msk[p, 0:len(lo_list)] = 1.0
                msk[p, L + 2:L + 2 + len(hi_list)] = 1.0
                if nd < LOA:
                    Alo[L, p] = nd
                    msk[p, D] = 1.0              # adm: dst row from view A
                else:
                    Ahi[0, p] = nd - MIDBASE
            si_list.append(_wrap16(Alo.reshape(-1)))
            si_list.append(_wrap16(Ahi.reshape(-1)))
            m_list.append(msk)
        siB_parts.append(np.concatenate(si_list, axis=1))
        mB_parts.append(np.concatenate(m_list, axis=1))
    siB = np.stack(siB_parts)                       # [8, 128, WB]
    mB = np.stack(mB_parts).astype(np.float32)      # [8, 128, sum(D+2)]

    # ---- C binning: sort by deg; one table of pair units ----
    nodesC = np.argsort(deg, kind="stable")
    binsC = nodesC.reshape(NBINS, P)
    DsC = []
    for g in range(BLOCKS_PER_CORE):
        nodes = binsC[g * 8:(g + 1) * 8].reshape(-1)
        DsC.append(int(deg[nodes].max()))

    siC_parts, mC_parts = [], []
    for k in range(N_CORES):
        si_list, m_list = [], []
        for j in range(BLOCKS_PER_CORE):
            De = DsC[j]
            nodes = binsC[j * 8 + k]
            A = np.zeros((De + 1, P), np.int64)     # De edge chunks + dst chunk
            val = np.zeros((P, De + 1), np.float32)
            par = np.zeros((P, De + 1), np.float32)
            for p in range(P):
                nd = nodes[p]
                es = src_s[estart[nd]:estart[nd + 1]]
                A[:len(es), p] = es >> 1
                val[p, 0:len(es)] = 1.0
                par[p, 0:len(es)] = (es & 1).astype(np.float32)
                A[De, p] = nd >> 1
                par[p, De] = float(nd & 1)           # dst col: valid stays 0
            si_list.append(_wrap16(A.reshape(-1)))
            m_list.append(np.concatenate([val, par], axis=1))
        siC_parts.append(np.concatenate(si_list, axis=1))
        mC_parts.append(np.concatenate(m_list, axis=1))
    siC = np.stack(siC_parts)
    mC = np.stack(mC_parts).astype(np.float32)

    return dict(LsB=LsB, HsB=HsB, DsC=DsC, binsB=binsB, binsC=binsC,
                siB=siB, mB=mB, siC=siC, mC=mC)


# ------------------------------- NEFF builders -------------------------------

def build_neff_a(reps=1):
    nc = bacc.Bacc()
    xT = nc.dram_tensor("xT", [P, NODES_PER_CORE], F32, kind="ExternalInput")
    w1e = nc.dram_tensor("w1e", [P, 144], F32, kind="ExternalInput")
    g_out = nc.dram_tensor("g_out", [P, BLOCKS_PER_CORE, 144], F16,
                           kind="ExternalOutput")
    ntiles = NODES_PER_CORE // P
    with tile.TileContext(nc) as tc:
        with tc.tile_pool(name="sbuf", bufs=2) as pool, \
             tc.tile_pool(name="psum", bufs=4, space="PSUM") as pp:
            w1t = pool.tile([P, 144], F32)
            nc.sync.dma_start(w1t[:], w1e[:])

            def body():
                xt = pool.tile([P, NODES_PER_CORE], F32, tag="xt", name="xt")
                nc.sync.dma_start(xt[:], xT[:, :])
                gt = pool.tile([P, ntiles, 144], F16, tag="gt", name="gt")
                for t in range(ntiles):
                    ps = pp.tile([P, 144], F32, tag="ps", space="PSUM", name="ps")
                    nc.tensor.matmul(out=ps[:], lhsT=xt[:, t * P:(t + 1) * P],
                                     rhs=w1t[:], start=True, stop=True)
                    nc.vector.tensor_copy(out=gt[:, t, :], in_=ps[:])
                nc.sync.dma_start(g_out[:, :, :], gt[:])

            if reps == 1:
                body()
            else:
                with tc.For_i(0, reps, 1):
                    body()
    nc.finalize()
    return nc


def _gather_calls(nc, out_tile, c0, nchunks, table_ap, si_tile, w0, row, qrr):
    """Issue dma_gather calls (split at GMAX) covering nchunks chunks of
    out_tile starting at chunk c0, indices from si_tile columns w0...."""
    n = nchunks * P
    done = 0
    while done < n:
        cnt = min(GMAX, n - done)
        nc.gpsimd.dma_gather(
            out_ap=out_tile[:, c0 + done // P:c0 + (done + cnt) // P, :],
            in_ap=table_ap,
            idxs_ap=si_tile[:, w0 + done // 16:w0 + (done + cnt) // 16],
            num_idxs=cnt, num_idxs_reg=cnt, elem_size=row,
            queue_num=qrr())
        done += cnt


def _tree_reduce(nc, pool, wx, nmax, nsl, width, tag):
    """Sum wx[:, 0:nsl, 0:width] over the slot axis -> [P, width] f32.
    Halving tree with contiguous halves: a[0:h] += a[h:2h]."""
    cur = wx
    n = nsl
    buf = pool.tile([P, (nmax + 1) // 2, width], F16, tag=tag + "_pp")
    while n > 2:
        half = n // 2
        nc.vector.tensor_tensor(out=buf[:, 0:half, :],
                                in0=cur[:, 0:half, :],
                                in1=cur[:, half:2 * half, :],
                                op=mybir.AluOpType.add)
        if n % 2:
            nc.vector.tensor_copy(out=buf[:, half, :], in_=cur[:, n - 1, :])
        cur, buf = buf, cur
        n = (n + 1) // 2
    out = pool.tile([P, width], F32, tag=tag + "_out")
    if n == 2:
        nc.vector.tensor_tensor(out=out[:], in0=cur[:, 0, :], in1=cur[:, 1, :],
                                op=mybir.AluOpType.add)
    else:
        nc.vector.tensor_copy(out=out[:], in_=cur[:, 0, :])
    return out


def build_neff_b(cfg, reps=1):
    nc = bacc.Bacc(num_swdge_queues=NQ)
    LsB, HsB = cfg["LsB"], cfg["HsB"]
    WB = cfg["siB"].shape[2]
    WM = cfg["mB"].shape[2]
    Dmax = max(L + H + 2 for L, H in zip(LsB, HsB))

    g_d = nc.dram_tensor("g", [N_PAD, GROW], F16, kind="ExternalInput")
    si_d = nc.dram_tensor("si", [P, WB], I16, kind="ExternalInput")
    m_d = nc.dram_tensor("m", [P, WM], F32, kind="ExternalInput")
    w2e_d = nc.dram_tensor("w2e", [P, 18], F32, kind="ExternalInput")
    b1_d = nc.dram_tensor("b1r", [P, P], F32, kind="ExternalInput")
    ident_d = nc.dram_tensor("ident", [P, P], F32, kind="ExternalInput")
    g2_out = nc.dram_tensor("g2_out", [BLOCKS_PER_CORE, P, 18], F16,
                            kind="ExternalOutput")
    gA = g_d[0:LOA, :]
    gB = g_d[MIDBASE:N_PAD, :]

    qctr = [0]

    def qrr():
        qctr[0] = (qctr[0] + 1) % NQ
        return qctr[0]

    with tile.TileContext(nc) as tc:
        with tc.tile_pool(name="sbuf", bufs=3) as pool, \
             tc.tile_pool(name="psum", bufs=2, space="PSUM") as pp:
            mshift = pool.tile([P, 1], F32)
            nc.gpsimd.memset(mshift[:], -M_SHIFT)
            w2t = pool.tile([P, 18], F32)
            nc.sync.dma_start(w2t[:], w2e_d[:])
            b1t = pool.tile([P, P], F32)
            nc.sync.dma_start(b1t[:], b1_d[:])
            identt = pool.tile([P, P], F32)
            nc.sync.dma_start(identt[:], ident_d[:])

            def body():
                woff = 0
                moff = 0
                for j in range(BLOCKS_PER_CORE):
                    L, H = LsB[j], HsB[j]
                    D = L + H + 2
                    si = pool.tile([P, Dmax * 8], I16, tag="si")
                    nwr = D * 8                      # (D*128)/16 index cols
                    nc.sync.dma_start(si[:, 0:nwr], si_d[:, woff:woff + nwr])
                    mt = pool.tile([P, Dmax + 2], F32, tag="mt")
                    nc.sync.dma_start(mt[:, 0:D + 2], m_d[:, moff:moff + D + 2])

                    X = pool.tile([P, Dmax, GROW], F16, tag="X")
                    _gather_calls(nc, X, 0, L + 1, gA, si, 0, GROW, qrr)
                    _gather_calls(nc, X, L + 1, H + 1, gB, si, (L + 1) * 8,
                                  GROW, qrr)

                    # dst row select: rd = XB + adm*(XA - XB), cols 0:144
                    adm = mt[:, D:D + 1]
                    rdd = pool.tile([P, 144], F16, tag="rdd")
                    nc.vector.tensor_tensor(
                        out=rdd[:], in0=X[:, L, 0:144], in1=X[:, L + 1, 0:144],
                        op=mybir.AluOpType.subtract)
                    rd = pool.tile([P, 144], F16, tag="rd")
                    nc.vector.tensor_tensor(
                        out=rd[:], in0=rdd[:],
                        in1=adm.to_broadcast([P, 144]),
                        op=mybir.AluOpType.mult)
                    nc.vector.tensor_tensor(out=rd[:], in0=rd[:],
                                            in1=X[:, L + 1, 0:144],
                                            op=mybir.AluOpType.add)

                    # per-edge attention
                    t_t = pool.tile([P, Dmax, HEADS], F32, tag="t")
                    nc.vector.tensor_tensor(
                        out=t_t[:, 0:D, :], in0=X[:, 0:D, 128:136],
                        in1=rd[:, None, 136:144].to_broadcast([P, D, HEADS]),
                        op=mybir.AluOpType.add)
                    e1 = pool.tile([P, Dmax, HEADS], F16, tag="e1")
                    nc.scalar.activation(e1[:, 0:D, :], t_t[:, 0:D, :],
                                         mybir.ActivationFunctionType.Exp,
                                         bias=mshift[:])
                    e2 = pool.tile([P, Dmax, HEADS], F16, tag="e2")
                    nc.scalar.activation(e2[:, 0:D, :], t_t[:, 0:D, :],
                                         mybir.ActivationFunctionType.Exp,
                                         bias=mshift[:], scale=NEG)
                    we = pool.tile([P, Dmax, HEADS], F16, tag="we")
                    nc.vector.tensor_tensor(out=we[:, 0:D, :], in0=e1[:, 0:D, :],
                                            in1=e2[:, 0:D, :],
                                            op=mybir.AluOpType.max)
                    nc.vector.tensor_tensor(
                        out=we[:, 0:D, :], in0=we[:, 0:D, :],
                        in1=mt[:, 0:D, None].to_broadcast([P, D, HEADS]),
                        op=mybir.AluOpType.mult)

                    wx = pool.tile([P, Dmax, 136], F16, tag="wx")
                    nc.vector.tensor_tensor(
                        out=wx[:, 0:D, 0:128].rearrange("p c (h k) -> p c h k", k=HID),
                        in0=X[:, 0:D, 0:128].rearrange("p c (h k) -> p c h k", k=HID),
                        in1=we[:, 0:D, :, None].to_broadcast([P, D, HEADS, HID]),
                        op=mybir.AluOpType.mult)
                    nc.vector.tensor_copy(out=wx[:, 0:D, 128:136], in_=we[:, 0:D, :])

                    acc = _tree_reduce(nc, pool, wx, Dmax, D, 136, "trB")

                    recip = pool.tile([P, HEADS], F32, tag="recip")
                    nc.vector.reciprocal(recip[:], acc[:, 128:136])
                    o1 = pool.tile([P, P], F32, tag="o1")
                    nc.vector.tensor_tensor(
                        out=o1[:].rearrange("p (h k) -> p h k", k=HID),
                        in0=acc[:, 0:128].rearrange("p (h k) -> p h k", k=HID),
                        in1=recip[:, :, None].to_broadcast([P, HEADS, HID]),
                        op=mybir.AluOpType.mult)
                    nc.vector.tensor_tensor(out=o1[:], in0=o1[:], in1=b1t[:],
                                            op=mybir.AluOpType.add)
                    vmin = pool.tile([P, P], F32, tag="vmin")
                    nc.vector.tensor_scalar(out=vmin[:], in0=o1[:], scalar1=0.0,
                                            scalar2=None, op0=mybir.AluOpType.min)
                    ev = pool.tile([P, P], F32, tag="ev")
                    nc.scalar.activation(ev[:], vmin[:],
                                         mybir.ActivationFunctionType.Exp)
                    elu = pool.tile([P, P], F32, tag="elu")
                    nc.vector.tensor_scalar(out=elu[:], in0=o1[:], scalar1=0.0,
                                            scalar2=None, op0=mybir.AluOpType.max)
                    nc.vector.tensor_tensor(out=elu[:], in0=elu[:], in1=ev[:],
                                            op=mybir.AluOpType.add)
                    nc.vector.tensor_scalar(out=elu[:], in0=elu[:], scalar1=-1.0,
                                            scalar2=None, op0=mybir.AluOpType.add)
                    eTp = pp.tile([P, P], F32, tag="eTp", space="PSUM")
                    nc.tensor.transpose(out=eTp[:], in_=elu[:], identity=identt[:])
                    eT = pool.tile([P, P], F32, tag="eT")
                    nc.vector.tensor_copy(out=eT[:], in_=eTp[:])
                    g2p = pp.tile([P, 18], F32, tag="g2p", space="PSUM")
                    nc.tensor.matmul(out=g2p[:], lhsT=eT[:], rhs=w2t[:],
                                     start=True, stop=True)
                    g2t = pool.tile([P, 18], F16, tag="g2t")
                    nc.vector.tensor_copy(out=g2t[:], in_=g2p[:])
                    nc.sync.dma_start(g2_out[j], g2t[:])

                    woff += D * 8
                    moff += D + 2

            if reps == 1:
                body()
            else:
                with tc.For_i(0, reps, 1):
                    body()
    nc.finalize()
    return nc


def build_neff_c(cfg, reps=1):
    nc = bacc.Bacc(num_swdge_queues=NQ)
    DsC = cfg["DsC"]
    WC = cfg["siC"].shape[2]
    WM = cfg["mC"].shape[2]
    Dmax = max(DsC) + 1

    g4_d = nc.dram_tensor("g4", [NUNIT, G2UNIT], F16, kind="ExternalInput")
    si_d = nc.dram_tensor("si", [P, WC], I16, kind="ExternalInput")
    m_d = nc.dram_tensor("m", [P, WM], F32, kind="ExternalInput")
    b2_d = nc.dram_tensor("b2r", [P, CLASSES], F32, kind="ExternalInput")
    out_d = nc.dram_tensor("out2", [BLOCKS_PER_CORE, P, CLASSES], F32,
                           kind="ExternalOutput")

    qctr = [0]

    def qrr():
        qctr[0] = (qctr[0] + 1) % NQ
        return qctr[0]

    with tile.TileContext(nc) as tc:
        with tc.tile_pool(name="sbuf", bufs=3) as pool, \
             tc.tile_pool(name="psum", bufs=2, space="PSUM") as pp:
            mshift = pool.tile([P, 1], F32)
            nc.gpsimd.memset(mshift[:], -M_SHIFT)
            b2t = pool.tile([P, CLASSES], F32)
            nc.sync.dma_start(b2t[:], b2_d[:])

            def body():
                woff = 0
                moff = 0
                for j in range(BLOCKS_PER_CORE):
                    De = DsC[j]
                    D = De + 1                      # edge chunks + dst chunk
                    si = pool.tile([P, Dmax * 8], I16, tag="si")
                    nc.sync.dma_start(si[:, 0:D * 8], si_d[:, woff:woff + D * 8])
                    mt = pool.tile([P, 2 * Dmax], F32, tag="mt")
                    nc.sync.dma_start(mt[:, 0:2 * D], m_d[:, moff:moff + D * 2])

                    X = pool.tile([P, Dmax, G2UNIT], F16, tag="X")
                    _gather_calls(nc, X, 0, D, g4_d[:, :], si, 0, G2UNIT, qrr)

                    # sub-row select by parity: e = Xlo + par*(Xhi - Xlo)
                    par = mt[:, D:2 * D]
                    esd = pool.tile([P, Dmax, 18], F16, tag="esd")
                    nc.vector.tensor_tensor(
                        out=esd[:, 0:D, :], in0=X[:, 0:D, G2SUB:G2SUB + 18],
                        in1=X[:, 0:D, 0:18], op=mybir.AluOpType.subtract)
                    es = pool.tile([P, Dmax, 18], F16, tag="es")
                    nc.vector.tensor_tensor(
                        out=es[:, 0:D, :], in0=esd[:, 0:D, :],
                        in1=par[:, :, None].to_broadcast([P, D, 18]),
                        op=mybir.AluOpType.mult)
                    nc.vector.tensor_tensor(out=es[:, 0:D, :], in0=es[:, 0:D, :],
                                            in1=X[:, 0:D, 0:18],
                                            op=mybir.AluOpType.add)

                    t_t = pool.tile([P, Dmax, 1], F32, tag="t")
                    nc.vector.tensor_tensor(
                        out=t_t[:, 0:D, :], in0=es[:, 0:D, 16:17],
                        in1=es[:, De:De + 1, 17:18].to_broadcast([P, D, 1]),
                        op=mybir.AluOpType.add)
                    e1 = pool.tile([P, Dmax, 1], F16, tag="e1")
                    nc.scalar.activation(e1[:, 0:D, :], t_t[:, 0:D, :],
                                         mybir.ActivationFunctionType.Exp,
                                         bias=mshift[:])
                    e2 = pool.tile([P, Dmax, 1], F16, tag="e2")
                    nc.scalar.activation(e2[:, 0:D, :], t_t[:, 0:D, :],
                                         mybir.ActivationFunctionType.Exp,
                                         bias=mshift[:], scale=NEG)
                    we = pool.tile([P, Dmax, 1], F16, tag="we")
                    nc.vector.tensor_tensor(out=we[:, 0:D, :], in0=e1[:, 0:D, :],
                                            in1=e2[:, 0:D, :],
                                            op=mybir.AluOpType.max)
                    nc.vector.tensor_tensor(out=we[:, 0:D, :], in0=we[:, 0:D, :],
                                            in1=mt[:, 0:D, None],
                                            op=mybir.AluOpType.mult)

                    wx = pool.tile([P, Dmax, 17], F16, tag="wx")
                    nc.vector.tensor_tensor(
                        out=wx[:, 0:D, 0:16], in0=es[:, 0:D, 0:16],
                        in1=we[:, 0:D, :].to_broadcast([P, D, 16]),
                        op=mybir.AluOpType.mult)
                    nc.vector.tensor_copy(out=wx[:, 0:D, 16:17], in_=we[:, 0:D, :])

                    acc = _tree_reduce(nc, pool, wx, Dmax, D, 17, "trC")

                    recip = pool.tile([P, 1], F32, tag="recip")
                    nc.vector.reciprocal(recip[:], acc[:, 16:17])
                    o2 = pool.tile([P, CLASSES], F32, tag="o2")
                    nc.vector.tensor_tensor(
                        out=o2[:], in0=acc[:, 0:16],
                        in1=recip[:].to_broadcast([P, CLASSES]),
                        op=mybir.AluOpType.mult)
                    nc.vector.tensor_tensor(out=o2[:], in0=o2[:], in1=b2t[:],
                                            op=mybir.AluOpType.add)
                    nc.sync.dma_start(out_d[j], o2[:])

                    woff += D * 8
                    moff += D * 2

            if reps == 1:
                body()
            else:
                with tc.For_i(0, reps, 1):
                    body()
    nc.finalize()
    return nc


# ------------------------------ runner plumbing ------------------------------

def make_runner(nc, n_cores=N_CORES):
    """Cached shard_map runner for a finalized Bass module."""
    import jax
    from jax.sharding import Mesh, PartitionSpec
    from jax.experimental.shard_map import shard_map
    from concourse.bass2jax import _bass_exec_p, install_neuronx_cc_hook, partition_id_tensor

    install_neuronx_cc_hook()
    partition_name = nc.partition_id_tensor.name if nc.partition_id_tensor else None
    in_names, out_names, out_avals = [], [], []
    for alloc in nc.m.functions[0].allocations:
        if not isinstance(alloc, mybir.MemoryLocationSet):
            continue
        name = alloc.memorylocations[0].name
        if alloc.kind == "ExternalInput":
            if name != partition_name:
                in_names.append(name)
        elif alloc.kind == "ExternalOutput":
            out_names.append(name)
            out_avals.append(jax.core.ShapedArray(tuple(alloc.tensor_shape),
                                                  mybir.dt.np(alloc.dtype)))
    n_params = len(in_names)
    all_names = in_names + out_names + ([partition_name] if partition_name else [])

    def _body(*args):
        operands = list(args)
        if partition_name is not None:
            operands.append(partition_id_tensor())
        return tuple(_bass_exec_p.bind(
            *operands, out_avals=tuple(out_avals), in_names=tuple(all_names),
            out_names=tuple(out_names), lowering_input_output_aliases=(),
            sim_require_finite=False, sim_require_nnan=False, nc=nc))

    devices = jax.devices()[:n_cores]
    mesh = Mesh(np.asarray(devices), ("core",))
    sharded = jax.jit(
        shard_map(_body, mesh=mesh,
                  in_specs=(PartitionSpec("core"),) * (n_params + len(out_names)),
                  out_specs=(PartitionSpec("core"),) * len(out_names),
                  check_rep=False),
        keep_unused=True)

    import jax as _jax
    from jax.sharding import NamedSharding

    _dev_args = {}

    def run(in_maps, key=None, raw=False):
        if key is not None and key in _dev_args:
            args = _dev_args[key]
        else:
            concat_in = [np.concatenate([np.asarray(m[nm]) for m in in_maps], axis=0)
                         for nm in in_names]
            concat_zero = [np.zeros((n_cores * a.shape[0], *a.shape[1:]), a.dtype)
                           for a in out_avals]
            sh = NamedSharding(mesh, PartitionSpec("core"))
            args = [_jax.device_put(a, sh) for a in concat_in + concat_zero]
            _jax.block_until_ready(args)
            if key is not None:
                _dev_args[key] = args
        outs = sharded(*args)
        _jax.block_until_ready(outs)
        if raw:
            return outs
        return [
            {nm: np.asarray(outs[i]).reshape(n_cores, *out_avals[i].shape)[c]
             for i, nm in enumerate(out_names)}
            for c in range(n_cores)
        ]

    return run


def _get_compiled(key, builder):
    if key not in _cache:
        nc = builder()
        _cache[key] = make_runner(nc)
    return _cache[key]


# --------------------------------- kernel ------------------------------------

def kernel(x, edge_index, W1, a_src1, a_dst1, b1, W2, a_src2, a_dst2, b2):
    x = np.asarray(x, np.float32)
    edge_index = np.asarray(edge_index)
    W1 = np.asarray(W1, np.float32)
    W2 = np.asarray(W2, np.float32)
    a_src1 = np.asarray(a_src1, np.float32)
    a_dst1 = np.asarray(a_dst1, np.float32)
    a_src2 = np.asarray(a_src2, np.float32)
    a_dst2 = np.asarray(a_dst2, np.float32)
    b1 = np.asarray(b1, np.float32)
    b2 = np.asarray(b2, np.float32)

    cfg = host_prep(edge_index)

    As = np.zeros((P, HEADS), np.float32)
    Ad = np.zeros((P, HEADS), np.float32)
    for h in range(HEADS):
        As[h * HID:(h + 1) * HID, h] = a_src1[h]
        Ad[h * HID:(h + 1) * HID, h] = a_dst1[h]
    W1ext = np.concatenate([W1, W1 @ As, W1 @ Ad], 1).astype(np.float32)
    W2ext = np.concatenate([W2, W2 @ a_src2.T, W2 @ a_dst2.T], 1).astype(np.float32)
    ident = np.eye(P, dtype=np.float32)
    b1r = np.ascontiguousarray(np.broadcast_to(b1, (P, P))).astype(np.float32)
    b2r = np.ascontiguousarray(np.broadcast_to(b2, (P, CLASSES))).astype(np.float32)

    xT = np.zeros((P, N_PAD), np.float32)
    xT[:, :N] = x.T

    # ---- NEFF-A ----
    run_a = _get_compiled("A", build_neff_a)
    in_a = [{"xT": np.ascontiguousarray(xT[:, k * NODES_PER_CORE:(k + 1) * NODES_PER_CORE]),
             "w1e": W1ext} for k in range(N_CORES)]
    res_a = run_a(in_a)
    g_full = np.zeros((N_PAD, GROW), np.float16)
    for k in range(N_CORES):
        # g_out [128, 49, 144]: node k*6272 + t*128 + p at [p, t, :]
        blk = res_a[k]["g_out"].transpose(1, 0, 2).reshape(-1, 144)
        g_full[k * NODES_PER_CORE:(k + 1) * NODES_PER_CORE, 0:144] = blk

    # ---- NEFF-B ----
    keyb = ("B", tuple(cfg["LsB"]), tuple(cfg["HsB"]))
    run_b = _get_compiled(keyb, lambda: build_neff_b(cfg))
    in_b = [{"g": g_full, "si": cfg["siB"][k], "m": cfg["mB"][k],
             "w2e": W2ext, "b1r": b1r, "ident": ident} for k in range(N_CORES)]
    res_b = run_b(in_b)
    g2_full = np.zeros((N_PAD, 18), np.float32)
    for k in range(N_CORES):
        rows = cfg["binsB"].reshape(NBINS, P)[np.arange(BLOCKS_PER_CORE) * 8 + k]
        g2_full[rows.reshape(-1)] = res_b[k]["g2_out"].reshape(-1, 18).astype(np.float32)
    g2_full[~np.isfinite(g2_full).all(1)] = 0
    g2_full[N:] = 0

    # pair-packed table for C: unit u holds nodes 2u, 2u+1 at sub-offsets 0, 64
    g4 = np.zeros((NUNIT, G2UNIT), np.float16)
    g4[:, 0:18] = g2_full[0::2]
    g4[:, G2SUB:G2SUB + 18] = g2_full[1::2]

    # ---- NEFF-C ----
    keyc = ("C", tuple(cfg["DsC"]))
    run_c = _get_compiled(keyc, lambda: build_neff_c(cfg))
    in_c = [{"g4": g4, "si": cfg["siC"][k], "m": cfg["mC"][k],
             "b2r": b2r} for k in range(N_CORES)]
    res_c = run_c(in_c)

    out = np.zeros((N_PAD, CLASSES), np.float32)
    for k in range(N_CORES):
        rows = cfg["binsC"].reshape(NBINS, P)[np.arange(BLOCKS_PER_CORE) * 8 + k]
        out[rows.reshape(-1)] = res_c[k]["out2"].reshape(-1, CLASSES)

    global _last_cfg, _last_inputs
    _last_cfg = cfg
    _last_inputs = {"A": in_a, "B": in_b, "C": in_c}
    return out[:N].astype(np.float32)
